# revision 1
# baseline (speedup 1.0000x reference)
"""Longformer (chunked sliding-window) self-attention on 8 TRN2 NeuronCores.

Sharding: sequence-parallel. B=2, L=4096 -> 8 blocks of 1024 query tokens
(4 blocks per batch element), one block per core. Each core also receives a
512-token K/V halo (the previous chunk), so no cross-core communication is
needed. The first block of each batch gets a zero halo; halo keys are made
invalid not by an additive mask but by a per-key validity column in V (see
below), which drops them from both softmax numerator and denominator exactly
like the reference's -1e9 masking.

On-chip layout choices (per core):
  - x is passed pre-transposed (xT [D, NKV], bf16) so QKV projections read it
    as the natural [din-on-partitions] matmul operand. Weights are passed
    pre-transposed (W.T, [din, dout], bf16).
  - q, k are produced transposed ([d, tok], bf16); v is produced natural
    ([tok, d], bf16) with a validity column appended per head: 1 for valid
    keys, 0 for halo keys (the denominator trick + masking in one).
  - scores are computed transposed (k_tok on partitions); two heads of a pair
    share one [128, 1024] 2-bank PSUM tile (head 2u in array rows 0..63,
    head 2u+1 in rows 64..127), so one ScalarE exp covers both heads and the
    paired matmuls can run concurrently in distinct PE row groups.
  - av^T accumulates per head into a [65, 512] PSUM tile; row 64 is the
    softmax denominator (from the validity column).
  - softmax division is deferred and pipelined per (chunk, head pair):
    denominators are DMA-gathered to a [2, 512] tile, reciprocal'd, broadcast
    to [128, 512] with a K=2 selection matmul, and multiplied into ctx^T
    while later pairs are still in flight.
  - out-projection consumes ctx^T directly as lhsT. All matmul operands are
    bf16; every accumulation is fp32 in PSUM. Emission order releases
    dependencies finely (q/k/v stripes before the attention pairs that need
    them) so the ScalarE exp stream overlaps the projection matmuls.
"""

import numpy as np

B, L, D = 2, 4096, 1024
H, DH, W = 16, 64, 512
NCORES = 8
BLK = L // 4          # 1024 query tokens per core
NKV = BLK + W         # 1536 kv tokens (halo + own)
CHUNKS = BLK // W     # 2 chunks per core
KT = (2 * W) // 128   # 8 k-token tiles of 128 per chunk window

_CACHE = {}


def _build():
    import concourse.bacc as bacc
    import concourse.mybir as mybir
    import concourse.tile as tile

    f32 = mybir.dt.float32
    bf16 = mybir.dt.bfloat16
    AF = mybir.ActivationFunctionType

    nc = bacc.Bacc("TRN2", target_bir_lowering=False, debug=False,
                   num_devices=NCORES)

    xT = nc.dram_tensor("xT", [D, NKV], bf16, kind="ExternalInput").ap()
    wqT = nc.dram_tensor("wqT", [D, D], bf16, kind="ExternalInput").ap()
    wkT = nc.dram_tensor("wkT", [D, D], bf16, kind="ExternalInput").ap()
    wvT = nc.dram_tensor("wvT", [D, D], bf16, kind="ExternalInput").ap()
    woT = nc.dram_tensor("woT", [D, D], bf16, kind="ExternalInput").ap()
    bqr = nc.dram_tensor("bqr", [128, 8], f32, kind="ExternalInput").ap()
    bkr = nc.dram_tensor("bkr", [128, 8], f32, kind="ExternalInput").ap()
    bvrep = nc.dram_tensor("bvrep", [128, D], f32, kind="ExternalInput").ap()
    borep = nc.dram_tensor("borep", [128, D], f32, kind="ExternalInput").ap()
    vones = nc.dram_tensor("vones", [128, 12], f32, kind="ExternalInput").ap()
    sel16 = nc.dram_tensor("sel16", [16, 1024], bf16, kind="ExternalInput").ap()
    out = nc.dram_tensor("out", [BLK, D], f32, kind="ExternalOutput").ap()

    xT_r = xT.rearrange("(ko p) t -> p ko t", p=128)     # [128, 8, 1536]
    wq_r = wqT.rearrange("(ko p) d -> p ko d", p=128)    # [128, 8, 1024]
    wk_r = wkT.rearrange("(ko p) d -> p ko d", p=128)
    wv_r = wvT.rearrange("(ko p) d -> p ko d", p=128)
    wo_r = woT.rearrange("(ko p) d -> p ko d", p=128)
    out_r = out.rearrange("(to p) d -> p to d", p=128)   # [128, 8, 1024]

    with tile.TileContext(nc) as tc:
        with (
            tc.tile_pool(name="const", bufs=1) as constp,
            tc.tile_pool(name="xw", bufs=1) as xwp,
            tc.tile_pool(name="wts", bufs=3) as wp,
            tc.tile_pool(name="acts", bufs=1) as actp,
            tc.tile_pool(name="ptiles", bufs=5) as pp,
            tc.tile_pool(name="normp", bufs=3) as normp,
            tc.tile_pool(name="outs", bufs=3) as op,
            tc.tile_pool(name="psA", bufs=2, space="PSUM") as psA,
            tc.tile_pool(name="psS", bufs=2, space="PSUM") as psS,
            tc.tile_pool(name="psV", bufs=2, space="PSUM") as psV,
        ):
            # ---- first-needed inputs; big loads spread across DMA queues ----
            bq_sb = constp.tile([128, 8], f32)
            bk_sb = constp.tile([128, 8], f32)
            nc.scalar.dma_start(bq_sb[:], bqr[:])
            nc.scalar.dma_start(bk_sb[:], bkr[:])

            x_sb = xwp.tile([128, 8, NKV], bf16)         # 24 KB/part
            wq_sb = wp.tile([128, 8, D], bf16, tag="w")
            wk_sb = wp.tile([128, 8, D], bf16, tag="w")
            wv_sb = wp.tile([128, 8, D], bf16, tag="w")
            for ko in range(8):
                eng = nc.sync if ko % 2 == 0 else nc.scalar
                oth = nc.scalar if ko % 2 == 0 else nc.sync
                eng.dma_start(wq_sb[:, ko], wq_r[:, ko])
                oth.dma_start(x_sb[:, ko, 512:1024], xT_r[:, ko, 512:1024])
            for ko in range(8):
                nc.sync.dma_start(wv_sb[:, ko], wv_r[:, ko])
                nc.scalar.dma_start(x_sb[:, ko, 0:512], xT_r[:, ko, 0:512])
            x_mm = x_sb[:]

            bv_sb = constp.tile([128, D], f32)
            vones_sb = constp.tile([128, 12], f32)
            sel16_sb = constp.tile([16, 1024], bf16)
            nc.scalar.dma_start(bv_sb[:], bvrep[:])
            nc.scalar.dma_start(vones_sb[:], vones[:])
            for ko in range(8):
                nc.scalar.dma_start(x_sb[:, ko, 1024:1536],
                                    xT_r[:, ko, 1024:1536])
                nc.scalar.dma_start(wk_sb[:, ko], wk_r[:, ko])
            nc.scalar.dma_start(sel16_sb[:], sel16[:])

            # ---- persistent activations ----
            q_sb = actp.tile([128, 8, BLK], bf16, tag="q")    # q^T [d, tok]
            k_sb = actp.tile([128, 8, NKV], bf16, tag="k")    # k^T [d, tok]
            v_sb = actp.tile([128, 12, H * (DH + 1)], bf16, tag="v")
            ctx_sb = actp.tile([128, 8, BLK], bf16, tag="ctx")  # ctx^T unnorm
            # denominator slots: partition 32*(h%4), column block c*4 + h//4
            den_sb = actp.tile([128, 8, 512], f32, tag="den")
            den_v = den_sb[:].rearrange("p (c r) q -> p r c q", c=CHUNKS)
            den2_sb = actp.tile([16, BLK], f32, tag="den2")

            # validity column per head: 1 for valid keys, 0 for halo keys
            v_ones = v_sb[:].rearrange("p t (h e) -> p t h e", e=DH + 1)
            nc.vector.tensor_copy(
                v_ones[:, :, :, DH],
                vones_sb[:, :, None].to_broadcast([128, 12, H]),
            )

            def proj_qk(w_mm, dst, bias, xn, dn, borrow=False):
                """One 512-token stripe of a q^T/k^T projection: x columns
                [xn*512, ..), destination columns [dn*512, ..). borrow=True
                additionally cycles psV's (idle until attention) banks for
                deeper accumulation pipelining."""
                for m in range(8):
                    if borrow and m % 2 == 1:
                        ps = psV.tile([128, 512], f32, name="av_b", tag="av")
                    else:
                        ps = psA.tile([128, 512], f32, name="ps", tag="ps")
                    for ko in range(8):
                        nc.tensor.matmul(
                            ps[:],
                            w_mm[:, ko, m * 128:(m + 1) * 128],
                            x_mm[:, ko, xn * 512:(xn + 1) * 512],
                            start=(ko == 0), stop=(ko == 7),
                        )
                    nc.vector.tensor_scalar_add(
                        dst[:, m, dn * 512:dn * 512 + 512],
                        ps[:], bias[:, m:m + 1],
                    )

            def proj_v(w_mm, t, n, borrow=False):
                """One [128-token x 512-feature] tile of the v projection
                (+bias, validity zeroing of halo rows)."""
                if borrow and t % 2 == 1:
                    ps = psV.tile([128, 512], f32, name="av_b", tag="av")
                else:
                    ps = psA.tile([128, 512], f32, name="ps", tag="ps")
                for ko in range(8):
                    nc.tensor.matmul(
                        ps[:],
                        x_mm[:, ko, t * 128:(t + 1) * 128],
                        w_mm[:, ko, n * 512:(n + 1) * 512],
                        start=(ko == 0), stop=(ko == 7),
                    )
                dst = v_ones[:, t, n * 8:(n + 1) * 8, :DH]
                nc.vector.tensor_add(
                    dst,
                    ps[:].rearrange("p (h e) -> p h e", e=DH),
                    bv_sb[:, n * 512:(n + 1) * 512]
                    .rearrange("p (h e) -> p h e", e=DH),
                )
                if t < 4:
                    # halo rows must be exactly zero (incl. bias) so they
                    # vanish from the attention numerator on block-0 cores
                    nc.vector.tensor_scalar_mul(
                        dst, dst, vones_sb[:, t:t + 1])

            def attn_pair(c, u):
                """Chunk c, head pair (2u, 2u+1): scores share one 2-bank
                PSUM tile and one exp; av^T accumulates per head."""
                hs_e = (2 * u) * (DH + 1)
                hs_o = (2 * u + 1) * (DH + 1)
                av_e_f = psV.tile([128, 512], f32, name="av_e", tag="av")
                av_o_f = psV.tile([128, 512], f32, name="av_o", tag="av")
                av_e, av_o = av_e_f[:DH + 1], av_o_f[:DH + 1]
                for i in range(KT):
                    ksl = slice(c * 512 + i * 128, c * 512 + (i + 1) * 128)
                    qsl = slice(c * 512, (c + 1) * 512)
                    sps = psS.tile([128, 1024], f32, name="sps")
                    nc.tensor.matmul(sps[:, 0:512],
                                     k_sb[0:64, u, ksl], q_sb[0:64, u, qsl],
                                     start=True, stop=True)
                    nc.tensor.matmul(sps[:, 512:1024],
                                     k_sb[64:128, u, ksl], q_sb[64:128, u, qsl],
                                     start=True, stop=True)
                    p_t = pp.tile([128, 1024], bf16, tag="p")
                    nc.scalar.activation(p_t[:], sps[:], AF.Exp, scale=0.125)
                    nc.tensor.matmul(av_e[:],
                                     v_sb[:, 4 * c + i, hs_e:hs_e + DH + 1],
                                     p_t[:, 0:512],
                                     start=(i == 0), stop=(i == KT - 1))
                    nc.tensor.matmul(av_o[:],
                                     v_sb[:, 4 * c + i, hs_o:hs_o + DH + 1],
                                     p_t[:, 512:1024],
                                     start=(i == 0), stop=(i == KT - 1))
                for g, av in ((0, av_e), (1, av_o)):
                    h = 2 * u + g
                    nc.vector.tensor_copy(
                        ctx_sb[g * 64:g * 64 + 64, u, c * 512:(c + 1) * 512],
                        av[:DH],
                    )
                    nc.vector.tensor_copy(
                        den_sb[32 * (h % 4):32 * (h % 4) + 1,
                               c * 4 + h // 4, :],
                        av[DH:DH + 1],
                    )

            def gather_half(c, u):
                """DMA the pair's denominators into den2 rows; pure DMA, no
                engine contention with the surrounding attention."""
                for g in range(2):
                    h = 2 * u + g
                    nc.sync.dma_start(
                        den2_sb[h:h + 1, c * 512:(c + 1) * 512],
                        den_v[32 * (h % 4):32 * (h % 4) + 1, h // 4, c],
                    )

            def norm_cols(c):
                """Normalize ctx^T columns of chunk c: one batched bf16
                reciprocal, then per-d-tile selection-matmul broadcast +
                multiply. Chunk 0's pass overlaps chunk-1 attention."""
                recb = normp.tile([16, 512], bf16, tag="recb")
                with nc.allow_low_precision(
                        reason="softmax denominators are O(100); bf16 recip "
                               "error ~4e-3 is well inside the accuracy gate"):
                    nc.vector.reciprocal(
                        recb[:], den2_sb[:, c * 512:(c + 1) * 512])
                for m in range(8):
                    ps = psA.tile([128, 512], f32, name="ps", tag="ps")
                    nc.tensor.matmul(
                        ps[:], sel16_sb[:, m * 128:(m + 1) * 128], recb[:],
                        start=True, stop=True,
                    )
                    nc.vector.tensor_mul(
                        ctx_sb[:, m, c * 512:(c + 1) * 512],
                        ctx_sb[:, m, c * 512:(c + 1) * 512],
                        ps[:],
                    )

            # ---- phase schedule: coarse phases (keeps PE dense); chunk-0
            # attention overlaps the K n2 / V t8..11 projection tails ----
            proj_qk(wq_sb[:], q_sb, bq_sb, 1, 0, borrow=True)   # q 0..512
            proj_qk(wq_sb[:], q_sb, bq_sb, 2, 1, borrow=True)   # q 512..1024
            for t in range(8):                      # v tiles 0..7 (chunk 0)
                proj_v(wv_sb[:], t, 0, borrow=True)
                proj_v(wv_sb[:], t, 1, borrow=True)
            proj_qk(wk_sb[:], k_sb, bk_sb, 0, 0, borrow=True)   # k 0..512
            proj_qk(wk_sb[:], k_sb, bk_sb, 1, 1, borrow=True)   # k 512..1024

            for u in range(8):                      # attention, chunk 0
                attn_pair(0, u)
                gather_half(0, u)

            proj_qk(wk_sb[:], k_sb, bk_sb, 2, 2)    # k cols 1024..1536
            for t in range(8, 12):                  # v tiles 8..11
                proj_v(wv_sb[:], t, 0)
                proj_v(wv_sb[:], t, 1)

            wo_sb = wp.tile([128, 8, D], bf16, tag="w")
            for ko in range(8):
                nc.sync.dma_start(wo_sb[:, ko], wo_r[:, ko])
            bo_sb = constp.tile([128, D], f32)
            nc.scalar.dma_start(bo_sb[:], borep[:])

            # ---- output projection body (to-tile granularity) ----
            ctx_mm = ctx_sb[:]
            wo_mm = wo_sb[:]

            def out_proj(to_range):
                for to in to_range:
                    for n in range(2):
                        ps = psA.tile([128, 512], f32, name="ps", tag="ps")
                        for ko in range(8):
                            nc.tensor.matmul(
                                ps[:],
                                ctx_mm[:, ko, to * 128:(to + 1) * 128],
                                wo_mm[:, ko, n * 512:(n + 1) * 512],
                                start=(ko == 0), stop=(ko == 7),
                            )
                        o_t = op.tile([128, 512], f32, tag="o")
                        nc.vector.tensor_add(o_t[:], ps[:],
                                             bo_sb[:, n * 512:(n + 1) * 512])
                        nc.sync.dma_start(
                            out_r[:, to, n * 512:(n + 1) * 512], o_t[:])

            norm_cols(0)   # chunk-0 softmax division; overlaps chunk 1
            for u in range(8):                      # attention, chunk 1
                attn_pair(1, u)
                gather_half(1, u)
            # out rows 0..511 are chunk-0 tokens: their lhsT columns are
            # already normalized, so this half of the output projection
            # fills PE idle slots inside the exp-bound chunk-1 window
            out_proj(range(0, 4))
            norm_cols(1)   # chunk-1 softmax division
            out_proj(range(4, 8))

    nc.compile()
    return nc


def _host_prep(x, Wq, bq, Wk, bk, Wv, bv, Wo, bo):
    import ml_dtypes

    bf = ml_dtypes.bfloat16
    x = np.ascontiguousarray(np.asarray(x, dtype=np.float32))
    # sel16[h, m*128 + p] = 1 iff d-row p of d-tile m belongs to head h
    sel16 = np.zeros((16, 1024), np.float32)
    for h in range(16):
        m, g = h // 2, h % 2
        sel16[h, m * 128 + g * 64: m * 128 + (g + 1) * 64] = 1.0
    mats = {
        "wqT": np.ascontiguousarray(np.asarray(Wq, np.float32).T.astype(bf)),
        "wkT": np.ascontiguousarray(np.asarray(Wk, np.float32).T.astype(bf)),
        "wvT": np.ascontiguousarray(np.asarray(Wv, np.float32).T.astype(bf)),
        "woT": np.ascontiguousarray(np.asarray(Wo, np.float32).T.astype(bf)),
        "bqr": np.ascontiguousarray(
            np.asarray(bq, np.float32).reshape(8, 128).T),
        "bkr": np.ascontiguousarray(
            np.asarray(bk, np.float32).reshape(8, 128).T),
        "bvrep": np.ascontiguousarray(
            np.tile(np.asarray(bv, np.float32)[None, :], (128, 1))),
        "borep": np.ascontiguousarray(
            np.tile(np.asarray(bo, np.float32)[None, :], (128, 1))),
        "sel16": sel16.astype(bf),
    }

    in_maps = []
    for core in range(NCORES):
        b, j = core // 4, core % 4
        start = j * BLK
        xkv = np.zeros((NKV, D), np.float32)
        lo = start - W
        if lo < 0:
            xkv[W:] = x[b, start:start + BLK]
        else:
            xkv[:] = x[b, lo:start + BLK]
        vo = np.ones((128, 12), np.float32)
        if j == 0:
            vo[:, 0:4] = 0.0         # halo keys (tokens 0..511) are invalid
        im = dict(mats)
        im["xT"] = np.ascontiguousarray(xkv.T.astype(bf))
        im["vones"] = vo
        in_maps.append(im)
    return in_maps


def kernel(x, Wq, bq, Wk, bk, Wv, bv, Wo, bo):
    from concourse.bass_utils import run_bass_kernel_spmd

    if "nc" not in _CACHE:
        _CACHE["nc"] = _build()
    nc = _CACHE["nc"]

    in_maps = _host_prep(x, Wq, bq, Wk, bk, Wv, bv, Wo, bo)
    res = run_bass_kernel_spmd(nc, in_maps, list(range(NCORES)))

    out = np.empty((B, L, D), np.float32)
    for core in range(NCORES):
        b, j = core // 4, core % 4
        out[b, j * BLK:(j + 1) * BLK] = res.results[core]["out"]
    return out



# revision 2
# speedup vs baseline: 1.0151x; 1.0151x over previous
"""Longformer (chunked sliding-window) self-attention on 8 TRN2 NeuronCores.

Sharding: sequence-parallel. B=2, L=4096 -> 8 blocks of 1024 query tokens
(4 blocks per batch element), one block per core. Each core also receives a
512-token K/V halo (the previous chunk), so no cross-core communication is
needed. The first block of each batch gets a zero halo; halo keys are made
invalid not by an additive mask but by a per-key validity column in V, which
drops them from both softmax numerator and denominator exactly like the
reference's -1e9 masking.

On-chip layout choices (per core):
  - x is passed pre-transposed (xT [D, NKV], bf16); weights pre-transposed
    (W.T, [din, dout], bf16).
  - q, k are produced transposed ([d, tok], bf16); v natural ([tok, d], bf16)
    with a validity column appended per head (1 valid / 0 halo).
  - scores are computed transposed (k_tok on psum partitions); two heads of a
    pair share one [128, 1024] 2-bank PSUM tile so one ScalarE exp covers both.
  - A*V runs in NATURAL orientation: lhsT = p^T tile [k,128q] (stationary),
    rhs = v tile [k, 65] (64 dims + validity column) -> psum [128q, 65]
    accumulated over the 8 k-tiles. This halves the PE cost vs the transposed
    formulation (moving dim 65 instead of 512 per head) and lands the softmax
    denominator in column 64 of the same psum tile.
  - softmax division fuses into the psum drain: per-partition reciprocal of
    column 64, then one broadcast multiply per head writes normalized ctx
    (natural [tok, d], bf16) to SBUF. No selection matmuls, no denominator
    gather DMAs.
  - ctx is transposed for the output projection by XBAR DMA-transpose
    (SBUF->SBUF, one instruction per 128-token tile: out[p, ko, t] =
    in[t, ko*128+p]), costing no PE or DVE cycles.
  - All matmul operands are bf16; every accumulation is fp32 in PSUM.
    Emission order: attention starts once q stripe 0, k stripes 0-1 and the
    v n=0 half are projected; the remaining projections, the wo load and the
    chunk-0 output projection are interleaved into the exp-bound attention
    windows so the PE stays dense.
"""

import numpy as np

B, L, D = 2, 4096, 1024
H, DH, W = 16, 64, 512
NCORES = 8
BLK = L // 4          # 1024 query tokens per core
NKV = BLK + W         # 1536 kv tokens (halo + own)
CHUNKS = BLK // W     # 2 chunks per core
KT = (2 * W) // 128   # 8 k-token tiles of 128 per chunk window

_CACHE = {}


def _build():
    import concourse.bacc as bacc
    import concourse.mybir as mybir
    import concourse.tile as tile

    f32 = mybir.dt.float32
    bf16 = mybir.dt.bfloat16
    AF = mybir.ActivationFunctionType

    nc = bacc.Bacc("TRN2", target_bir_lowering=False, debug=False,
                   num_devices=NCORES)

    xT = nc.dram_tensor("xT", [D, NKV], bf16, kind="ExternalInput").ap()
    wqT = nc.dram_tensor("wqT", [D, D], bf16, kind="ExternalInput").ap()
    wkT = nc.dram_tensor("wkT", [D, D], bf16, kind="ExternalInput").ap()
    wvT = nc.dram_tensor("wvT", [D, D], bf16, kind="ExternalInput").ap()
    woT = nc.dram_tensor("woT", [D, D], bf16, kind="ExternalInput").ap()
    bqr = nc.dram_tensor("bqr", [128, 8], f32, kind="ExternalInput").ap()
    bkr = nc.dram_tensor("bkr", [128, 8], f32, kind="ExternalInput").ap()
    bvrep = nc.dram_tensor("bvrep", [128, D], f32, kind="ExternalInput").ap()
    borep = nc.dram_tensor("borep", [128, D], f32, kind="ExternalInput").ap()
    vones = nc.dram_tensor("vones", [128, 12], f32, kind="ExternalInput").ap()
    out = nc.dram_tensor("out", [BLK, D], f32, kind="ExternalOutput").ap()

    xT_r = xT.rearrange("(ko p) t -> p ko t", p=128)     # [128, 8, 1536]
    wq_r = wqT.rearrange("(ko p) d -> p ko d", p=128)    # [128, 8, 1024]
    wk_r = wkT.rearrange("(ko p) d -> p ko d", p=128)
    wv_r = wvT.rearrange("(ko p) d -> p ko d", p=128)
    wo_r = woT.rearrange("(ko p) d -> p ko d", p=128)
    out_r = out.rearrange("(to p) d -> p to d", p=128)   # [128, 8, 1024]

    with tile.TileContext(nc) as tc:
        with (
            tc.tile_pool(name="const", bufs=1) as constp,
            tc.tile_pool(name="xw", bufs=1) as xwp,
            tc.tile_pool(name="wts", bufs=3) as wp,
            tc.tile_pool(name="wo", bufs=1) as wop,
            tc.tile_pool(name="acts", bufs=1) as actp,
            tc.tile_pool(name="ptiles", bufs=3) as pp,
            tc.tile_pool(name="normp", bufs=4) as normp,
            tc.tile_pool(name="outs", bufs=2) as op,
            tc.tile_pool(name="psA", bufs=2, space="PSUM") as psA,
            tc.tile_pool(name="psS", bufs=2, space="PSUM") as psS,
            tc.tile_pool(name="psV", bufs=1, space="PSUM") as psV,
        ):
            # ---- inputs, ordered so q-stripe-0 dependencies land first ----
            bq_sb = constp.tile([128, 8], f32)
            bk_sb = constp.tile([128, 8], f32)
            nc.scalar.dma_start(bq_sb[:], bqr[:])
            nc.scalar.dma_start(bk_sb[:], bkr[:])

            x_sb = xwp.tile([128, 8, NKV], bf16)         # 24 KB/part
            wq_sb = wp.tile([128, 8, D], bf16, tag="w")
            wk_sb = wp.tile([128, 8, D], bf16, tag="w")
            wv_sb = wp.tile([128, 8, D], bf16, tag="w")
            for ko in range(8):
                nc.sync.dma_start(wq_sb[:, ko], wq_r[:, ko])
                nc.scalar.dma_start(x_sb[:, ko, 512:1024],
                                    xT_r[:, ko, 512:1024])
            for ko in range(8):
                nc.sync.dma_start(wk_sb[:, ko], wk_r[:, ko])
                nc.scalar.dma_start(x_sb[:, ko, 0:512], xT_r[:, ko, 0:512])
            for ko in range(8):
                nc.sync.dma_start(wv_sb[:, ko], wv_r[:, ko])
                nc.scalar.dma_start(x_sb[:, ko, 1024:1536],
                                    xT_r[:, ko, 1024:1536])
            x_mm = x_sb[:]

            bv_sb = constp.tile([128, D], f32)
            vones_sb = constp.tile([128, 12], f32)
            nc.scalar.dma_start(bv_sb[:], bvrep[:])
            nc.scalar.dma_start(vones_sb[:], vones[:])

            # wo/bo have no producers; load in the background on sync
            wo_sb = wop.tile([128, 8, D], bf16)
            bo_sb = constp.tile([128, D], f32)
            for ko in range(8):
                nc.sync.dma_start(wo_sb[:, ko], wo_r[:, ko])
            nc.sync.dma_start(bo_sb[:], borep[:])

            # ---- persistent activations ----
            q_sb = actp.tile([128, 8, BLK], bf16, tag="q")    # q^T [d, tok]
            k_sb = actp.tile([128, 8, NKV], bf16, tag="k")    # k^T [d, tok]
            v_sb = actp.tile([128, 12, H * (DH + 1)], bf16, tag="v")
            # ctx natural [q-part, j, head, dh], one chunk at a time
            ctxn_sb = actp.tile([128, 4, H, DH], bf16, tag="ctxn")
            ctxT_sb = actp.tile([128, 8, BLK], bf16, tag="ctxT")  # ctx^T

            # validity column per head: 1 for valid keys, 0 for halo keys
            v_ones = v_sb[:].rearrange("p t (h e) -> p t h e", e=DH + 1)
            nc.vector.tensor_copy(
                v_ones[:, :, :, DH],
                vones_sb[:, :, None].to_broadcast([128, 12, H]),
            )

            def proj_qk(w_mm, dst, bias, xn, dn, ms):
                """m-tile groups [ms] of a 512-token stripe of a q^T/k^T
                projection: x columns [xn*512, ..), dst columns [dn*512, ..)."""
                for m in ms:
                    ps = psA.tile([128, 512], f32, name="ps", tag="ps")
                    for ko in range(8):
                        nc.tensor.matmul(
                            ps[:],
                            w_mm[:, ko, m * 128:(m + 1) * 128],
                            x_mm[:, ko, xn * 512:(xn + 1) * 512],
                            start=(ko == 0), stop=(ko == 7),
                        )
                    nc.vector.tensor_scalar_add(
                        dst[:, m, dn * 512:dn * 512 + 512],
                        ps[:], bias[:, m:m + 1],
                    )

            def proj_v(t, n):
                """One [128-token x 512-feature] tile of the v projection
                (+bias, validity zeroing of halo rows)."""
                ps = psA.tile([128, 512], f32, name="ps", tag="ps")
                for ko in range(8):
                    nc.tensor.matmul(
                        ps[:],
                        x_mm[:, ko, t * 128:(t + 1) * 128],
                        wv_sb[:, ko, n * 512:(n + 1) * 512],
                        start=(ko == 0), stop=(ko == 7),
                    )
                dst = v_ones[:, t, n * 8:(n + 1) * 8, :DH]
                nc.vector.tensor_add(
                    dst,
                    ps[:].rearrange("p (h e) -> p h e", e=DH),
                    bv_sb[:, n * 512:(n + 1) * 512]
                    .rearrange("p (h e) -> p h e", e=DH),
                )
                if t < 4:
                    # halo rows must be exactly zero (incl. bias) so they
                    # vanish from the attention numerator on block-0 cores
                    nc.vector.tensor_scalar_mul(
                        dst, dst, vones_sb[:, t:t + 1])

            def attn_pair(c, u):
                """Chunk c, head pair (2u, 2u+1): transposed scores share one
                2-bank PSUM tile / one exp; A*V accumulates naturally into
                [128q, 65] per (q-subtile, head), denominator in column 64."""
                hs_e = (2 * u) * (DH + 1)
                hs_o = (2 * u + 1) * (DH + 1)
                av_e = psV.tile([128, 4, DH + 1], f32, name="av_e", tag="av_e")
                av_o = psV.tile([128, 4, DH + 1], f32, name="av_o", tag="av_o")
                qsl = slice(c * 512, (c + 1) * 512)
                for i in range(KT):
                    ksl = slice(c * 512 + i * 128, c * 512 + (i + 1) * 128)
                    sps = psS.tile([128, 1024], f32, name="sps")
                    nc.tensor.matmul(sps[:, 0:512],
                                     k_sb[0:64, u, ksl], q_sb[0:64, u, qsl],
                                     start=True, stop=True)
                    nc.tensor.matmul(sps[:, 512:1024],
                                     k_sb[64:128, u, ksl], q_sb[64:128, u, qsl],
                                     start=True, stop=True)
                    p_t = pp.tile([128, 1024], bf16, tag="p")
                    nc.scalar.activation(p_t[:], sps[:], AF.Exp, scale=0.125)
                    for j in range(4):
                        nc.tensor.matmul(
                            av_e[:, j],
                            p_t[:, j * 128:(j + 1) * 128],
                            v_sb[:, 4 * c + i, hs_e:hs_e + DH + 1],
                            start=(i == 0), stop=(i == KT - 1),
                        )
                        nc.tensor.matmul(
                            av_o[:, j],
                            p_t[:, 512 + j * 128:512 + (j + 1) * 128],
                            v_sb[:, 4 * c + i, hs_o:hs_o + DH + 1],
                            start=(i == 0), stop=(i == KT - 1),
                        )
                # drain: normalize by the denominator column while copying
                # psum -> sbuf (natural ctx, bf16)
                for g, av in ((0, av_e), (1, av_o)):
                    h = 2 * u + g
                    rec = normp.tile([128, 4, 1], f32, tag="rec")
                    nc.vector.reciprocal(rec[:], av[:, :, DH:DH + 1])
                    nc.vector.tensor_mul(
                        ctxn_sb[:, :, h, :],
                        av[:, :, 0:DH],
                        rec[:].to_broadcast([128, 4, DH]),
                    )

            def transp(c, j):
                """ctx natural q-tile j -> ctx^T columns, via XBAR DMA:
                out[p, ko, t] = in[t, ko*128 + p]."""
                nc.sync.dma_start_transpose(
                    ctxT_sb[:, :, c * 512 + j * 128:c * 512 + (j + 1) * 128],
                    ctxn_sb[:, j].rearrange("p h e -> p (h e)"),
                )

            def out_proj(to):
                for n in range(2):
                    ps = psA.tile([128, 512], f32, name="ps", tag="ps")
                    for ko in range(8):
                        nc.tensor.matmul(
                            ps[:],
                            ctxT_sb[:, ko, to * 128:(to + 1) * 128],
                            wo_sb[:, ko, n * 512:(n + 1) * 512],
                            start=(ko == 0), stop=(ko == 7),
                        )
                    o_t = op.tile([128, 512], f32, tag="o")
                    nc.vector.tensor_add(o_t[:], ps[:],
                                         bo_sb[:, n * 512:(n + 1) * 512])
                    nc.scalar.dma_start(
                        out_r[:, to, n * 512:(n + 1) * 512], o_t[:])

            # ---- prelude: just enough for chunk-0 attention ----
            proj_qk(wq_sb[:], q_sb, bq_sb, 1, 0, range(8))   # q stripe 0
            proj_qk(wk_sb[:], k_sb, bk_sb, 0, 0, range(8))   # k stripe 0
            proj_qk(wk_sb[:], k_sb, bk_sb, 1, 1, range(8))   # k stripe 1
            for t in range(8):                               # v heads 0..7
                proj_v(t, 0)

            # ---- chunk-0 attention; remaining projections fill PE slack ----
            fill = [
                [("v", t, 1) for t in range(0, 4)],
                [("v", t, 1) for t in range(4, 8)],
                [("k2", m) for m in range(0, 4)],
                [("k2", m) for m in range(4, 8)],
                [("v", t, 0) for t in range(8, 12)],
                [("v", t, 1) for t in range(8, 12)],
                [("q1", m) for m in range(0, 4)],
                [("q1", m) for m in range(4, 8)],
            ]
            for u in range(8):
                attn_pair(0, u)
                for item in fill[u]:
                    if item[0] == "v":
                        proj_v(item[1], item[2])
                    elif item[0] == "k2":
                        proj_qk(wk_sb[:], k_sb, bk_sb, 2, 2, [item[1]])
                    else:
                        proj_qk(wq_sb[:], q_sb, bq_sb, 2, 1, [item[1]])

            for j in range(4):
                transp(0, j)

            # ---- chunk-1 attention; chunk-0 output projection overlaps ----
            for u in range(8):
                attn_pair(1, u)
                if u < 4:
                    out_proj(u)

            for j in range(4):
                transp(1, j)
            for to in range(4, 8):
                out_proj(to)

    nc.compile()
    return nc


def _host_prep(x, Wq, bq, Wk, bk, Wv, bv, Wo, bo):
    import ml_dtypes

    bf = ml_dtypes.bfloat16
    x = np.ascontiguousarray(np.asarray(x, dtype=np.float32))
    mats = {
        "wqT": np.ascontiguousarray(np.asarray(Wq, np.float32).T.astype(bf)),
        "wkT": np.ascontiguousarray(np.asarray(Wk, np.float32).T.astype(bf)),
        "wvT": np.ascontiguousarray(np.asarray(Wv, np.float32).T.astype(bf)),
        "woT": np.ascontiguousarray(np.asarray(Wo, np.float32).T.astype(bf)),
        "bqr": np.ascontiguousarray(
            np.asarray(bq, np.float32).reshape(8, 128).T),
        "bkr": np.ascontiguousarray(
            np.asarray(bk, np.float32).reshape(8, 128).T),
        "bvrep": np.ascontiguousarray(
            np.tile(np.asarray(bv, np.float32)[None, :], (128, 1))),
        "borep": np.ascontiguousarray(
            np.tile(np.asarray(bo, np.float32)[None, :], (128, 1))),
    }

    in_maps = []
    for core in range(NCORES):
        b, j = core // 4, core % 4
        start = j * BLK
        xkv = np.zeros((NKV, D), np.float32)
        lo = start - W
        if lo < 0:
            xkv[W:] = x[b, start:start + BLK]
        else:
            xkv[:] = x[b, lo:start + BLK]
        vo = np.ones((128, 12), np.float32)
        if j == 0:
            vo[:, 0:4] = 0.0         # halo keys (tokens 0..511) are invalid
        im = dict(mats)
        im["xT"] = np.ascontiguousarray(xkv.T.astype(bf))
        im["vones"] = vo
        in_maps.append(im)
    return in_maps


def kernel(x, Wq, bq, Wk, bk, Wv, bv, Wo, bo):
    from concourse.bass_utils import run_bass_kernel_spmd

    if "nc" not in _CACHE:
        _CACHE["nc"] = _build()
    nc = _CACHE["nc"]

    in_maps = _host_prep(x, Wq, bq, Wk, bk, Wv, bv, Wo, bo)
    res = run_bass_kernel_spmd(nc, in_maps, list(range(NCORES)))

    out = np.empty((B, L, D), np.float32)
    for core in range(NCORES):
        b, j = core // 4, core % 4
        out[b, j * BLK:(j + 1) * BLK] = res.results[core]["out"]
    return out


# revision 3
# speedup vs baseline: 1.1036x; 1.0872x over previous
"""Longformer (chunked sliding-window) self-attention on 8 TRN2 NeuronCores.

Sharding: sequence-parallel. B=2, L=4096 -> 8 blocks of 1024 query tokens
(4 blocks per batch element), one block per core. Each core also receives a
512-token K/V halo (the previous chunk), so no cross-core communication is
needed. The first block of each batch gets a zero halo; halo keys are made
invalid not by an additive mask but by a per-key validity column in V, which
drops them from both softmax numerator and denominator exactly like the
reference's -1e9 masking.

On-chip layout choices (per core):
  - x is passed pre-transposed (xT [D, NKV], bf16); weights pre-transposed
    (W.T, [din, dout], bf16).
  - q, k are produced transposed ([d, tok], bf16); v natural ([tok, d], bf16)
    with a validity column appended per head (1 valid / 0 halo).
  - scores are computed transposed (k_tok on psum partitions); two heads of a
    pair share one [128, 1024] 2-bank PSUM tile so one ScalarE exp covers both.
  - A*V runs in NATURAL orientation: lhsT = p^T tile [k,128q] (stationary),
    rhs = v tile [k, 65] (64 dims + validity column) -> psum [128q, 65]
    accumulated over the 8 k-tiles. This halves the PE cost vs the transposed
    formulation (moving dim 65 instead of 512 per head) and lands the softmax
    denominator in column 64 of the same psum tile.
  - softmax division fuses into the psum drain: per-partition reciprocal of
    column 64, then one broadcast multiply per head writes normalized ctx
    (natural [tok, d], bf16) to SBUF. No selection matmuls, no denominator
    gather DMAs.
  - ctx is transposed for the output projection by XBAR DMA-transpose
    (SBUF->SBUF, one [128q x 128d] instruction per (q-tile, head pair),
    emitted right after that pair's drain), costing no PE or DVE cycles.
  - Scheduling: the attention exp stream is the per-chunk pacing item, so all
    non-prelude projection work (q stripe 1, k stripe 2, v tiles 8-11, the
    n=1 half of v, and the chunk-0 output projection) is chopped into
    single-matmul thunks and dripped into the attention loop between k-tile
    iterations, keeping the PE dense (which also keeps it at full p-state).
"""

from collections import deque

import numpy as np

B, L, D = 2, 4096, 1024
H, DH, W = 16, 64, 512
NCORES = 8
BLK = L // 4          # 1024 query tokens per core
NKV = BLK + W         # 1536 kv tokens (halo + own)
CHUNKS = BLK // W     # 2 chunks per core
KT = (2 * W) // 128   # 8 k-token tiles of 128 per chunk window

_CACHE = {}


def _build():
    import concourse.bacc as bacc
    import concourse.mybir as mybir
    import concourse.tile as tile

    f32 = mybir.dt.float32
    bf16 = mybir.dt.bfloat16
    AF = mybir.ActivationFunctionType

    nc = bacc.Bacc("TRN2", target_bir_lowering=False, debug=False,
                   num_devices=NCORES)

    xT = nc.dram_tensor("xT", [D, NKV], bf16, kind="ExternalInput").ap()
    wqT = nc.dram_tensor("wqT", [D, D], bf16, kind="ExternalInput").ap()
    wkT = nc.dram_tensor("wkT", [D, D], bf16, kind="ExternalInput").ap()
    wvT = nc.dram_tensor("wvT", [D, D], bf16, kind="ExternalInput").ap()
    woT = nc.dram_tensor("woT", [D, D], bf16, kind="ExternalInput").ap()
    bqr = nc.dram_tensor("bqr", [128, 8], f32, kind="ExternalInput").ap()
    bkr = nc.dram_tensor("bkr", [128, 8], f32, kind="ExternalInput").ap()
    bvrep = nc.dram_tensor("bvrep", [128, D], f32, kind="ExternalInput").ap()
    borep = nc.dram_tensor("borep", [128, D], f32, kind="ExternalInput").ap()
    vones = nc.dram_tensor("vones", [128, 12], f32, kind="ExternalInput").ap()
    out = nc.dram_tensor("out", [BLK, D], f32, kind="ExternalOutput").ap()

    xT_r = xT.rearrange("(ko p) t -> p ko t", p=128)     # [128, 8, 1536]
    wq_r = wqT.rearrange("(ko p) d -> p ko d", p=128)    # [128, 8, 1024]
    wk_r = wkT.rearrange("(ko p) d -> p ko d", p=128)
    wv_r = wvT.rearrange("(ko p) d -> p ko d", p=128)
    wo_r = woT.rearrange("(ko p) d -> p ko d", p=128)
    out_r = out.rearrange("(to p) d -> p to d", p=128)   # [128, 8, 1024]

    with tile.TileContext(nc) as tc:
        with (
            tc.tile_pool(name="const", bufs=1) as constp,
            tc.tile_pool(name="xw", bufs=1) as xwp,
            tc.tile_pool(name="wts", bufs=3) as wp,
            tc.tile_pool(name="wo", bufs=1) as wop,
            tc.tile_pool(name="acts", bufs=1) as actp,
            tc.tile_pool(name="ptiles", bufs=3) as pp,
            tc.tile_pool(name="normp", bufs=4) as normp,
            tc.tile_pool(name="outs", bufs=2) as op,
            tc.tile_pool(name="psA", bufs=2, space="PSUM") as psA,
            tc.tile_pool(name="psS", bufs=2, space="PSUM") as psS,
            tc.tile_pool(name="psV", bufs=1, space="PSUM") as psV,
        ):
            # ---- inputs, ordered by first need ----
            bq_sb = constp.tile([128, 8], f32)
            bk_sb = constp.tile([128, 8], f32)
            vones_sb = constp.tile([128, 12], f32)
            nc.scalar.dma_start(bq_sb[:], bqr[:])
            nc.scalar.dma_start(bk_sb[:], bkr[:])
            nc.scalar.dma_start(vones_sb[:], vones[:])

            x_sb = xwp.tile([128, 8, NKV], bf16)         # 24 KB/part
            wq_sb = wp.tile([128, 8, D], bf16, tag="w")
            wk_sb = wp.tile([128, 8, D], bf16, tag="w")
            wv_sb = wp.tile([128, 8, D], bf16, tag="w")
            # sync: weights in use order; scalar: x stripes in use order
            for ko in range(8):
                nc.sync.dma_start(wq_sb[:, ko], wq_r[:, ko])
                nc.scalar.dma_start(x_sb[:, ko, 512:1024],
                                    xT_r[:, ko, 512:1024])
            for ko in range(8):
                nc.sync.dma_start(wk_sb[:, ko], wk_r[:, ko])
                nc.scalar.dma_start(x_sb[:, ko, 0:512], xT_r[:, ko, 0:512])
            for ko in range(8):
                nc.sync.dma_start(wv_sb[:, ko], wv_r[:, ko])
                nc.scalar.dma_start(x_sb[:, ko, 1024:1536],
                                    xT_r[:, ko, 1024:1536])
            x_mm = x_sb[:]

            bv_sb = constp.tile([128, D], f32)
            nc.scalar.dma_start(bv_sb[:], bvrep[:])

            # wo/bo have no producers; load in the background on sync
            wo_sb = wop.tile([128, 8, D], bf16)
            bo_sb = constp.tile([128, D], f32)
            for ko in range(8):
                nc.sync.dma_start(wo_sb[:, ko], wo_r[:, ko])
            nc.sync.dma_start(bo_sb[:], borep[:])

            # ---- persistent activations ----
            q_sb = actp.tile([128, 8, BLK], bf16, tag="q")    # q^T [d, tok]
            k_sb = actp.tile([128, 8, NKV], bf16, tag="k")    # k^T [d, tok]
            v_sb = actp.tile([128, 12, H * (DH + 1)], bf16, tag="v")
            # ctx natural [q-part, j, head, dh], one chunk at a time
            ctxn_sb = actp.tile([128, 4, H, DH], bf16, tag="ctxn")
            ctxT_sb = actp.tile([128, 8, BLK], bf16, tag="ctxT")  # ctx^T

            v_ones = v_sb[:].rearrange("p t (h e) -> p t h e", e=DH + 1)

            # ---- thunk-granular work queue dripped into attention ----
            work = deque()

            def drip(n):
                for _ in range(n):
                    if work:
                        work.popleft()()

            def g_proj_qk(w_mm, dst, bias, xn, dn, m):
                """8 matmul thunks for one m-tile of a q^T/k^T stripe; the
                last thunk also adds the bias (per-partition scalar)."""
                box = {}

                def mk(ko):
                    def f():
                        if ko == 0:
                            box["ps"] = psA.tile([128, 512], f32,
                                                 name="ps", tag="ps")
                        nc.tensor.matmul(
                            box["ps"][:],
                            w_mm[:, ko, m * 128:(m + 1) * 128],
                            x_mm[:, ko, xn * 512:(xn + 1) * 512],
                            start=(ko == 0), stop=(ko == 7),
                        )
                        if ko == 7:
                            nc.vector.tensor_scalar_add(
                                dst[:, m, dn * 512:dn * 512 + 512],
                                box["ps"][:], bias[:, m:m + 1],
                            )
                    return f
                return [mk(ko) for ko in range(8)]

            def g_proj_v(t, n):
                """8 matmul thunks for one [128-token x 8-head] v tile; the
                last adds bias and zeroes halo rows."""
                box = {}

                def mk(ko):
                    def f():
                        if ko == 0:
                            box["ps"] = psA.tile([128, 512], f32,
                                                 name="ps", tag="ps")
                        nc.tensor.matmul(
                            box["ps"][:],
                            x_mm[:, ko, t * 128:(t + 1) * 128],
                            wv_sb[:, ko, n * 512:(n + 1) * 512],
                            start=(ko == 0), stop=(ko == 7),
                        )
                        if ko == 7:
                            dst = v_ones[:, t, n * 8:(n + 1) * 8, :DH]
                            nc.vector.tensor_add(
                                dst,
                                box["ps"][:].rearrange("p (h e) -> p h e",
                                                       e=DH),
                                bv_sb[:, n * 512:(n + 1) * 512]
                                .rearrange("p (h e) -> p h e", e=DH),
                            )
                            if t < 4:
                                nc.vector.tensor_scalar_mul(
                                    dst, dst, vones_sb[:, t:t + 1])
                    return f
                return [mk(ko) for ko in range(8)]

            def g_out_proj(to, n):
                """8 matmul thunks for one [128-token x 512] out tile; the
                last adds bias and stores."""
                box = {}

                def mk(ko):
                    def f():
                        if ko == 0:
                            box["ps"] = psA.tile([128, 512], f32,
                                                 name="ps", tag="ps")
                        nc.tensor.matmul(
                            box["ps"][:],
                            ctxT_sb[:, ko, to * 128:(to + 1) * 128],
                            wo_sb[:, ko, n * 512:(n + 1) * 512],
                            start=(ko == 0), stop=(ko == 7),
                        )
                        if ko == 7:
                            o_t = op.tile([128, 512], f32, tag="o")
                            nc.vector.tensor_add(
                                o_t[:], box["ps"][:],
                                bo_sb[:, n * 512:(n + 1) * 512])
                            nc.scalar.dma_start(
                                out_r[:, to, n * 512:(n + 1) * 512], o_t[:])
                    return f
                return [mk(ko) for ko in range(8)]

            def attn_pair(c, u, drips):
                """Chunk c, head pair (2u, 2u+1). drips[i] fill thunks are
                emitted after each k-tile's matmuls to keep the PE dense
                through the exp-paced stretch."""
                hs_e = (2 * u) * (DH + 1)
                hs_o = (2 * u + 1) * (DH + 1)
                av_e = psV.tile([128, 4, DH + 1], f32, name="av_e", tag="av_e")
                av_o = psV.tile([128, 4, DH + 1], f32, name="av_o", tag="av_o")
                qsl = slice(c * 512, (c + 1) * 512)
                for i in range(KT):
                    ksl = slice(c * 512 + i * 128, c * 512 + (i + 1) * 128)
                    sps = psS.tile([128, 1024], f32, name="sps")
                    nc.tensor.matmul(sps[:, 0:512],
                                     k_sb[0:64, u, ksl], q_sb[0:64, u, qsl],
                                     start=True, stop=True)
                    nc.tensor.matmul(sps[:, 512:1024],
                                     k_sb[64:128, u, ksl], q_sb[64:128, u, qsl],
                                     start=True, stop=True)
                    p_t = pp.tile([128, 1024], bf16, tag="p")
                    nc.scalar.activation(p_t[:], sps[:], AF.Exp, scale=0.125)
                    for j in range(4):
                        nc.tensor.matmul(
                            av_e[:, j],
                            p_t[:, j * 128:(j + 1) * 128],
                            v_sb[:, 4 * c + i, hs_e:hs_e + DH + 1],
                            start=(i == 0), stop=(i == KT - 1),
                        )
                        nc.tensor.matmul(
                            av_o[:, j],
                            p_t[:, 512 + j * 128:512 + (j + 1) * 128],
                            v_sb[:, 4 * c + i, hs_o:hs_o + DH + 1],
                            start=(i == 0), stop=(i == KT - 1),
                        )
                    drip(drips[i])
                # drain: normalize by the denominator column while copying
                # psum -> sbuf (natural ctx, bf16)
                for g, av in ((0, av_e), (1, av_o)):
                    h = 2 * u + g
                    rec = normp.tile([128, 4, 1], f32, tag="rec")
                    nc.vector.reciprocal(rec[:], av[:, :, DH:DH + 1])
                    nc.vector.tensor_mul(
                        ctxn_sb[:, :, h, :],
                        av[:, :, 0:DH],
                        rec[:].to_broadcast([128, 4, DH]),
                    )
                # ctx^T columns for this pair via XBAR DMA: per q-tile j,
                # out[p, t] = in[t, p] over this pair's 128 d-columns
                for j in range(4):
                    nc.sync.dma_start_transpose(
                        ctxT_sb[:, u, c * 512 + j * 128:c * 512 + (j + 1) * 128],
                        ctxn_sb[:, j, 2 * u:2 * u + 2, :]
                        .rearrange("p h e -> p (h e)"),
                    )

            # ---- prelude: just enough for chunk-0 attention ----
            for m in range(8):                    # q stripe 0 (chunk-0 qs)
                for f in g_proj_qk(wq_sb[:], q_sb, bq_sb, 1, 0, m):
                    f()
            for m in range(8):                    # k stripe 1 (reuses x s1)
                for f in g_proj_qk(wk_sb[:], k_sb, bk_sb, 1, 1, m):
                    f()
            # validity column per head: 1 for valid keys, 0 for halo keys
            nc.vector.tensor_copy(
                v_ones[:, :, :, DH],
                vones_sb[:, :, None].to_broadcast([128, 12, H]),
            )
            for m in range(8):                    # k stripe 0 (halo)
                for f in g_proj_qk(wk_sb[:], k_sb, bk_sb, 0, 0, m):
                    f()
            for t in range(8):                    # v heads 0..7, kv tiles 0..7
                for f in g_proj_v(t, 0):
                    f()

            # ---- chunk-0 attention; fill queue: things chunk 1 needs ----
            for t in range(12):                   # v n=1 (t<8) + n=0 (t>=8)
                work.extend(g_proj_v(t % 8 + (8 if t >= 8 else 0),
                                     1 if t < 8 else 0))
            work.extend(g_proj_qk(wq_sb[:], q_sb, bq_sb, 2, 1, 0))  # q1 m0
            work.extend(g_proj_qk(wk_sb[:], k_sb, bk_sb, 2, 2, 0))  # k2 m0
            # 16 groups / 128 thunks; v n=1 t0..7 must land before pair 4
            d0 = [3, 3, 3, 3, 2, 2, 2, 2]
            for u in range(8):
                attn_pair(0, u, d0 if u < 4 else [2, 2, 2, 2, 2, 2, 2, 1])

            # ---- chunk-1 attention; fill queue: rest of q1/k2, v n=1
            # tiles 8-11, chunk-0 output projection ----
            for m in range(1, 4):
                work.extend(g_proj_qk(wq_sb[:], q_sb, bq_sb, 2, 1, m))
                work.extend(g_proj_qk(wk_sb[:], k_sb, bk_sb, 2, 2, m))
            for t in range(8, 12):
                work.extend(g_proj_v(t, 1))
            for m in range(4, 8):
                work.extend(g_proj_qk(wq_sb[:], q_sb, bq_sb, 2, 1, m))
                work.extend(g_proj_qk(wk_sb[:], k_sb, bk_sb, 2, 2, m))
            for to in range(4):
                for n in range(2):
                    work.extend(g_out_proj(to, n))
            d1 = [4, 4, 4, 4, 4, 4, 4, 4]
            for u in range(8):
                attn_pair(1, u, d1)
            while work:                           # flush any leftovers
                work.popleft()()

            # ---- chunk-1 output projection (tail) ----
            for to in range(4, 8):
                for n in range(2):
                    for f in g_out_proj(to, n):
                        f()

    nc.compile()
    return nc


def _host_prep(x, Wq, bq, Wk, bk, Wv, bv, Wo, bo):
    import ml_dtypes

    bf = ml_dtypes.bfloat16
    x = np.ascontiguousarray(np.asarray(x, dtype=np.float32))
    mats = {
        "wqT": np.ascontiguousarray(np.asarray(Wq, np.float32).T.astype(bf)),
        "wkT": np.ascontiguousarray(np.asarray(Wk, np.float32).T.astype(bf)),
        "wvT": np.ascontiguousarray(np.asarray(Wv, np.float32).T.astype(bf)),
        "woT": np.ascontiguousarray(np.asarray(Wo, np.float32).T.astype(bf)),
        "bqr": np.ascontiguousarray(
            np.asarray(bq, np.float32).reshape(8, 128).T),
        "bkr": np.ascontiguousarray(
            np.asarray(bk, np.float32).reshape(8, 128).T),
        "bvrep": np.ascontiguousarray(
            np.tile(np.asarray(bv, np.float32)[None, :], (128, 1))),
        "borep": np.ascontiguousarray(
            np.tile(np.asarray(bo, np.float32)[None, :], (128, 1))),
    }

    in_maps = []
    for core in range(NCORES):
        b, j = core // 4, core % 4
        start = j * BLK
        xkv = np.zeros((NKV, D), np.float32)
        lo = start - W
        if lo < 0:
            xkv[W:] = x[b, start:start + BLK]
        else:
            xkv[:] = x[b, lo:start + BLK]
        vo = np.ones((128, 12), np.float32)
        if j == 0:
            vo[:, 0:4] = 0.0         # halo keys (tokens 0..511) are invalid
        im = dict(mats)
        im["xT"] = np.ascontiguousarray(xkv.T.astype(bf))
        im["vones"] = vo
        in_maps.append(im)
    return in_maps


def kernel(x, Wq, bq, Wk, bk, Wv, bv, Wo, bo):
    from concourse.bass_utils import run_bass_kernel_spmd

    if "nc" not in _CACHE:
        _CACHE["nc"] = _build()
    nc = _CACHE["nc"]

    in_maps = _host_prep(x, Wq, bq, Wk, bk, Wv, bv, Wo, bo)
    res = run_bass_kernel_spmd(nc, in_maps, list(range(NCORES)))

    out = np.empty((B, L, D), np.float32)
    for core in range(NCORES):
        b, j = core // 4, core % 4
        out[b, j * BLK:(j + 1) * BLK] = res.results[core]["out"]
    return out


# revision 7
# speedup vs baseline: 1.1547x; 1.0464x over previous
"""Longformer (chunked sliding-window) self-attention on 8 TRN2 NeuronCores.

Sharding: sequence-parallel. B=2, L=4096 -> 8 blocks of 1024 query tokens
(4 blocks per batch element), one block per core. Each core also receives a
512-token K/V halo (the previous chunk), so no cross-core communication is
needed. The first block of each batch gets a zero halo; halo keys are made
invalid not by an additive mask but by a per-key validity column in V, which
drops them from both softmax numerator and denominator exactly like the
reference's -1e9 masking.

On-chip layout choices (per core):
  - x is passed pre-transposed (xT [D, NKV], bf16); weights pre-transposed
    (W.T, [din, dout], bf16).
  - q, k are produced transposed ([d, tok], bf16); v natural ([tok, d], bf16)
    with a validity column appended per head (1 valid / 0 halo).
  - scores are computed transposed (k_tok on psum partitions); two heads of a
    pair share one [128, 1024] 2-bank PSUM tile so one ScalarE exp covers both.
  - A*V runs in NATURAL orientation: lhsT = p^T tile [k,128q] (stationary),
    rhs = v tile [k, 65] (64 dims + validity column) -> psum [128q, 65]
    accumulated over the 8 k-tiles. This halves the PE cost vs the transposed
    formulation (moving dim 65 instead of 512 per head) and lands the softmax
    denominator in column 64 of the same psum tile.
  - softmax division fuses into the psum drain: per-partition reciprocal of
    column 64, then one broadcast multiply per head writes normalized ctx
    (natural [tok, d], bf16) to SBUF. No selection matmuls, no denominator
    gather DMAs.
  - ctx is transposed for the output projection by XBAR DMA-transpose
    (SBUF->SBUF, one [128q x 128d] instruction per (q-tile, head pair),
    emitted right after that pair's drain), costing no PE or DVE cycles.
  - Scheduling: the attention exp stream is the per-chunk pacing item, so all
    non-prelude projection work (q stripe 1, k stripe 2, v tiles 8-11, the
    n=1 half of v, and the chunk-0 output projection) is chopped into
    single-matmul thunks and dripped into the attention loop between k-tile
    iterations, keeping the PE dense (which also keeps it at full p-state).
"""

from collections import deque

import numpy as np

B, L, D = 2, 4096, 1024
H, DH, W = 16, 64, 512
NCORES = 8
BLK = L // 4          # 1024 query tokens per core
NKV = BLK + W         # 1536 kv tokens (halo + own)
CHUNKS = BLK // W     # 2 chunks per core
KT = (2 * W) // 128   # 8 k-token tiles of 128 per chunk window

_CACHE = {}


def _build():
    import concourse.bacc as bacc
    import concourse.mybir as mybir
    import concourse.tile as tile

    f32 = mybir.dt.float32
    bf16 = mybir.dt.bfloat16
    AF = mybir.ActivationFunctionType

    nc = bacc.Bacc("TRN2", target_bir_lowering=False, debug=False,
                   num_devices=NCORES)

    xT = nc.dram_tensor("xT", [D, NKV], bf16, kind="ExternalInput").ap()
    wqT = nc.dram_tensor("wqT", [D, D], bf16, kind="ExternalInput").ap()
    wkT = nc.dram_tensor("wkT", [D, D], bf16, kind="ExternalInput").ap()
    wvT = nc.dram_tensor("wvT", [D, D], bf16, kind="ExternalInput").ap()
    woT = nc.dram_tensor("woT", [D, D], bf16, kind="ExternalInput").ap()
    bqr = nc.dram_tensor("bqr", [128, 8], f32, kind="ExternalInput").ap()
    bkr = nc.dram_tensor("bkr", [128, 8], f32, kind="ExternalInput").ap()
    bvrep = nc.dram_tensor("bvrep", [128, D], f32, kind="ExternalInput").ap()
    borep = nc.dram_tensor("borep", [128, D], f32, kind="ExternalInput").ap()
    vones = nc.dram_tensor("vones", [128, 12], f32, kind="ExternalInput").ap()
    out = nc.dram_tensor("out", [BLK, D], f32, kind="ExternalOutput").ap()

    xT_r = xT.rearrange("(ko p) t -> p ko t", p=128)     # [128, 8, 1536]
    wq_r = wqT.rearrange("(ko p) d -> p ko d", p=128)    # [128, 8, 1024]
    wk_r = wkT.rearrange("(ko p) d -> p ko d", p=128)
    wv_r = wvT.rearrange("(ko p) d -> p ko d", p=128)
    wo_r = woT.rearrange("(ko p) d -> p ko d", p=128)
    out_r = out.rearrange("(to p) d -> p to d", p=128)   # [128, 8, 1024]

    with tile.TileContext(nc) as tc:
        with (
            tc.tile_pool(name="const", bufs=1) as constp,
            tc.tile_pool(name="xw", bufs=1) as xwp,
            tc.tile_pool(name="wts", bufs=3) as wp,
            tc.tile_pool(name="wo", bufs=1) as wop,
            tc.tile_pool(name="acts", bufs=1) as actp,
            tc.tile_pool(name="ptiles", bufs=3) as pp,
            tc.tile_pool(name="normp", bufs=4) as normp,
            tc.tile_pool(name="outs", bufs=2) as op,
            tc.tile_pool(name="psA", bufs=2, space="PSUM") as psA,
            tc.tile_pool(name="psS", bufs=2, space="PSUM") as psS,
            tc.tile_pool(name="psV", bufs=1, space="PSUM") as psV,
        ):
            # ---- inputs, ordered by first need ----
            bq_sb = constp.tile([128, 8], f32)
            bk_sb = constp.tile([128, 8], f32)
            vones_sb = constp.tile([128, 12], f32)
            nc.scalar.dma_start(bq_sb[:], bqr[:])

            x_sb = xwp.tile([128, 8, NKV], bf16)         # 24 KB/part
            wq_sb = wp.tile([128, 8, D], bf16, tag="w")
            wk_sb = wp.tile([128, 8, D], bf16, tag="w")
            wv_sb = wp.tile([128, 8, D], bf16, tag="w")
            # sync: weights in use order; scalar: x stripes in use order
            for ko in range(8):
                nc.sync.dma_start(wq_sb[:, ko], wq_r[:, ko])
                nc.scalar.dma_start(x_sb[:, ko, 512:1024],
                                    xT_r[:, ko, 512:1024])
            nc.scalar.dma_start(bk_sb[:], bkr[:])
            nc.scalar.dma_start(vones_sb[:], vones[:])
            for ko in range(8):
                nc.sync.dma_start(wk_sb[:, ko], wk_r[:, ko])
                nc.scalar.dma_start(x_sb[:, ko, 0:512], xT_r[:, ko, 0:512])
            for ko in range(8):
                nc.sync.dma_start(wv_sb[:, ko], wv_r[:, ko])
                nc.scalar.dma_start(x_sb[:, ko, 1024:1536],
                                    xT_r[:, ko, 1024:1536])
            x_mm = x_sb[:]

            bv_sb = constp.tile([128, D], f32)
            nc.scalar.dma_start(bv_sb[:], bvrep[:])

            # wo/bo have no producers; load in the background on sync
            wo_sb = wop.tile([128, 8, D], bf16)
            bo_sb = constp.tile([128, D], f32)
            for ko in range(8):
                nc.sync.dma_start(wo_sb[:, ko], wo_r[:, ko])
            nc.sync.dma_start(bo_sb[:], borep[:])

            # ---- persistent activations ----
            q_sb = actp.tile([128, 8, BLK], bf16, tag="q")    # q^T [d, tok]
            k_sb = actp.tile([128, 8, NKV], bf16, tag="k")    # k^T [d, tok]
            v_sb = actp.tile([128, 12, H * (DH + 1)], bf16, tag="v")
            # ctx natural [q-part, j, head, dh], one chunk at a time
            ctxn_sb = actp.tile([128, 4, H, DH], bf16, tag="ctxn")
            ctxT_sb = actp.tile([128, 8, BLK], bf16, tag="ctxT")  # ctx^T

            v_ones = v_sb[:].rearrange("p t (h e) -> p t h e", e=DH + 1)

            # ---- thunk-granular work queue dripped into attention ----
            work = deque()

            def drip(n):
                for _ in range(n):
                    if work:
                        work.popleft()()

            def g_proj_qk(w_mm, dst, bias, xn, dn, m):
                """8 matmul thunks for one m-tile of a q^T/k^T stripe; the
                last thunk also adds the bias (per-partition scalar)."""
                box = {}

                def mk(ko):
                    def f():
                        if ko == 0:
                            box["ps"] = psA.tile([128, 512], f32,
                                                 name="ps", tag="ps")
                        nc.tensor.matmul(
                            box["ps"][:],
                            w_mm[:, ko, m * 128:(m + 1) * 128],
                            x_mm[:, ko, xn * 512:(xn + 1) * 512],
                            start=(ko == 0), stop=(ko == 7),
                        )
                        if ko == 7:
                            nc.vector.tensor_scalar_add(
                                dst[:, m, dn * 512:dn * 512 + 512],
                                box["ps"][:], bias[:, m:m + 1],
                            )
                    return f
                return [mk(ko) for ko in range(8)]

            def g_proj_v(t, n):
                """8 matmul thunks for one [128-token x 8-head] v tile; the
                last adds bias and zeroes halo rows."""
                box = {}

                def mk(ko):
                    def f():
                        if ko == 0:
                            box["ps"] = psA.tile([128, 512], f32,
                                                 name="ps", tag="ps")
                        nc.tensor.matmul(
                            box["ps"][:],
                            x_mm[:, ko, t * 128:(t + 1) * 128],
                            wv_sb[:, ko, n * 512:(n + 1) * 512],
                            start=(ko == 0), stop=(ko == 7),
                        )
                        if ko == 7:
                            dst = v_ones[:, t, n * 8:(n + 1) * 8, :DH]
                            nc.vector.tensor_add(
                                dst,
                                box["ps"][:].rearrange("p (h e) -> p h e",
                                                       e=DH),
                                bv_sb[:, n * 512:(n + 1) * 512]
                                .rearrange("p (h e) -> p h e", e=DH),
                            )
                            if t < 4:
                                nc.vector.tensor_scalar_mul(
                                    dst, dst, vones_sb[:, t:t + 1])
                    return f
                return [mk(ko) for ko in range(8)]

            def g_out_proj(to, n):
                """8 matmul thunks for one [128-token x 512] out tile; the
                last adds bias and stores."""
                box = {}

                def mk(ko):
                    def f():
                        if ko == 0:
                            box["ps"] = psA.tile([128, 512], f32,
                                                 name="ps", tag="ps")
                        nc.tensor.matmul(
                            box["ps"][:],
                            ctxT_sb[:, ko, to * 128:(to + 1) * 128],
                            wo_sb[:, ko, n * 512:(n + 1) * 512],
                            start=(ko == 0), stop=(ko == 7),
                        )
                        if ko == 7:
                            o_t = op.tile([128, 512], f32, tag="o")
                            nc.vector.tensor_add(
                                o_t[:], box["ps"][:],
                                bo_sb[:, n * 512:(n + 1) * 512])
                            nc.scalar.dma_start(
                                out_r[:, to, n * 512:(n + 1) * 512], o_t[:])
                    return f
                return [mk(ko) for ko in range(8)]

            def attn_pair(c, u, drips):
                """Chunk c, head pair (2u, 2u+1). drips[i] fill thunks are
                emitted after each k-tile's score matmuls, and the A*V batch
                for k-tile i is deferred until after the (i+1)-th scores +
                fills, so the PE never waits on the exp stream."""
                hs_e = (2 * u) * (DH + 1)
                hs_o = (2 * u + 1) * (DH + 1)
                av_e = psV.tile([128, 4, DH + 1], f32, name="av_e", tag="av_e")
                av_o = psV.tile([128, 4, DH + 1], f32, name="av_o", tag="av_o")
                qsl = slice(c * 512, (c + 1) * 512)

                def av_batch(i, p_t):
                    for j in range(4):
                        nc.tensor.matmul(
                            av_e[:, j],
                            p_t[:, j * 128:(j + 1) * 128],
                            v_sb[:, 4 * c + i, hs_e:hs_e + DH + 1],
                            start=(i == 0), stop=(i == KT - 1),
                        )
                        nc.tensor.matmul(
                            av_o[:, j],
                            p_t[:, 512 + j * 128:512 + (j + 1) * 128],
                            v_sb[:, 4 * c + i, hs_o:hs_o + DH + 1],
                            start=(i == 0), stop=(i == KT - 1),
                        )

                prev = None
                for i in range(KT):
                    ksl = slice(c * 512 + i * 128, c * 512 + (i + 1) * 128)
                    sps = psS.tile([128, 1024], f32, name="sps")
                    nc.tensor.matmul(sps[:, 0:512],
                                     k_sb[0:64, u, ksl], q_sb[0:64, u, qsl],
                                     start=True, stop=True)
                    nc.tensor.matmul(sps[:, 512:1024],
                                     k_sb[64:128, u, ksl], q_sb[64:128, u, qsl],
                                     start=True, stop=True)
                    p_t = pp.tile([128, 1024], bf16, tag="p")
                    nc.scalar.activation(p_t[:], sps[:], AF.Exp, scale=0.125)
                    drip(drips[i])
                    if prev is not None:
                        av_batch(i - 1, prev)
                    prev = p_t
                av_batch(KT - 1, prev)
                # drain: normalize by the denominator column while copying
                # psum -> sbuf (natural ctx, bf16)
                for g, av in ((0, av_e), (1, av_o)):
                    h = 2 * u + g
                    rec = normp.tile([128, 4, 1], f32, tag="rec")
                    nc.vector.reciprocal(rec[:], av[:, :, DH:DH + 1])
                    nc.vector.tensor_mul(
                        ctxn_sb[:, :, h, :],
                        av[:, :, 0:DH],
                        rec[:].to_broadcast([128, 4, DH]),
                    )
                # ctx^T columns for this pair via XBAR DMA: per q-tile j,
                # out[p, t] = in[t, p] over this pair's 128 d-columns
                for j in range(4):
                    nc.sync.dma_start_transpose(
                        ctxT_sb[:, u, c * 512 + j * 128:c * 512 + (j + 1) * 128],
                        ctxn_sb[:, j, 2 * u:2 * u + 2, :]
                        .rearrange("p h e -> p (h e)"),
                    )

            # ---- prelude: just enough for chunk-0 attention ----
            for m in range(8):                    # q stripe 0 (chunk-0 qs)
                for f in g_proj_qk(wq_sb[:], q_sb, bq_sb, 1, 0, m):
                    f()
            for m in range(8):                    # k stripe 1 (reuses x s1)
                for f in g_proj_qk(wk_sb[:], k_sb, bk_sb, 1, 1, m):
                    f()
            # validity column per head: 1 for valid keys, 0 for halo keys
            nc.vector.tensor_copy(
                v_ones[:, :, :, DH],
                vones_sb[:, :, None].to_broadcast([128, 12, H]),
            )
            for m in range(8):                    # k stripe 0 (halo)
                for f in g_proj_qk(wk_sb[:], k_sb, bk_sb, 0, 0, m):
                    f()
            for t in range(8):                    # v heads 0..7, kv tiles 0..7
                for f in g_proj_v(t, 0):
                    f()

            # ---- chunk-0 attention; fill queue: things chunk 1 needs ----
            for t in range(12):                   # v n=1 (t<8) + n=0 (t>=8)
                work.extend(g_proj_v(t % 8 + (8 if t >= 8 else 0),
                                     1 if t < 8 else 0))
            work.extend(g_proj_qk(wq_sb[:], q_sb, bq_sb, 2, 1, 0))  # q1 m0
            work.extend(g_proj_qk(wk_sb[:], k_sb, bk_sb, 2, 2, 0))  # k2 m0
            # 14 groups / 112 thunks; v n=1 t0..7 must land before pair 4
            for u in range(8):
                attn_pair(0, u, [3] * 8 if u == 0 else [2] * 8)

            # ---- chunk-1 attention; fill queue: rest of q1/k2, v n=1
            # tiles 8-11, chunk-0 output projection ----
            for m in range(1, 4):
                work.extend(g_proj_qk(wq_sb[:], q_sb, bq_sb, 2, 1, m))
                work.extend(g_proj_qk(wk_sb[:], k_sb, bk_sb, 2, 2, m))
            for t in range(8, 12):
                work.extend(g_proj_v(t, 1))
            for m in range(4, 8):
                work.extend(g_proj_qk(wq_sb[:], q_sb, bq_sb, 2, 1, m))
                work.extend(g_proj_qk(wk_sb[:], k_sb, bk_sb, 2, 2, m))
            for to in range(4):
                for n in range(2):
                    work.extend(g_out_proj(to, n))
            # 208 thunks over 64 k-tile slots: 3.25/slot, spread evenly
            d1 = [3, 3, 3, 4, 3, 3, 3, 4]
            for u in range(8):
                attn_pair(1, u, d1)
            while work:                           # flush any leftovers
                work.popleft()()

            # ---- chunk-1 output projection (tail) ----
            for to in range(4, 8):
                for n in range(2):
                    for f in g_out_proj(to, n):
                        f()

    nc.compile()
    return nc


def _host_prep(x, Wq, bq, Wk, bk, Wv, bv, Wo, bo):
    import ml_dtypes

    bf = ml_dtypes.bfloat16
    x = np.ascontiguousarray(np.asarray(x, dtype=np.float32))
    mats = {
        "wqT": np.ascontiguousarray(np.asarray(Wq, np.float32).T.astype(bf)),
        "wkT": np.ascontiguousarray(np.asarray(Wk, np.float32).T.astype(bf)),
        "wvT": np.ascontiguousarray(np.asarray(Wv, np.float32).T.astype(bf)),
        "woT": np.ascontiguousarray(np.asarray(Wo, np.float32).T.astype(bf)),
        "bqr": np.ascontiguousarray(
            np.asarray(bq, np.float32).reshape(8, 128).T),
        "bkr": np.ascontiguousarray(
            np.asarray(bk, np.float32).reshape(8, 128).T),
        "bvrep": np.ascontiguousarray(
            np.tile(np.asarray(bv, np.float32)[None, :], (128, 1))),
        "borep": np.ascontiguousarray(
            np.tile(np.asarray(bo, np.float32)[None, :], (128, 1))),
    }

    in_maps = []
    for core in range(NCORES):
        b, j = core // 4, core % 4
        start = j * BLK
        xkv = np.zeros((NKV, D), np.float32)
        lo = start - W
        if lo < 0:
            xkv[W:] = x[b, start:start + BLK]
        else:
            xkv[:] = x[b, lo:start + BLK]
        vo = np.ones((128, 12), np.float32)
        if j == 0:
            vo[:, 0:4] = 0.0         # halo keys (tokens 0..511) are invalid
        im = dict(mats)
        im["xT"] = np.ascontiguousarray(xkv.T.astype(bf))
        im["vones"] = vo
        in_maps.append(im)
    return in_maps


def kernel(x, Wq, bq, Wk, bk, Wv, bv, Wo, bo):
    from concourse.bass_utils import run_bass_kernel_spmd

    if "nc" not in _CACHE:
        _CACHE["nc"] = _build()
    nc = _CACHE["nc"]

    in_maps = _host_prep(x, Wq, bq, Wk, bk, Wv, bv, Wo, bo)
    res = run_bass_kernel_spmd(nc, in_maps, list(range(NCORES)))

    out = np.empty((B, L, D), np.float32)
    for core in range(NCORES):
        b, j = core // 4, core % 4
        out[b, j * BLK:(j + 1) * BLK] = res.results[core]["out"]
    return out
